# revision 1
# baseline (speedup 1.0000x reference)
"""BetaTCVAE loss kernel for 8 TRN2 NeuronCores (Bass/Tile).

Math
----
reference:  out = (BETA-1)*tc + sum(kl)
  lp[i,j,d] = -0.5*((z_i - m_j)^2 * exp(-lv_j) + lv_j + LOG2PI)   (per dim d)
  log_qz_product[i] = sum_d logsumexp_j lp[i,j,d]
  log_qz[i]         = logsumexp_j sum_d lp[i,j,d]
  tc = mean_i(log_qz - log_qz_product)

Decomposition used here (per core, rows i sharded 256/core):
  * log_qz: S'[i,j] = sum_d(-0.5*w*z^2 + w*m*z - 0.5*(w*m^2+lv)) is a pair of
    [256x64]@[64x2048] matmuls plus a rank-1 term -> TensorEngine;
    log_qz[i] = logsumexp_j S'[i,j] - 32*LOG2PI.
  * log_qz_product: A[i,d] = sum_j q*exp(-0.5*w*(z-m)^2). With s=sqrt(w/2)
    the weight q = exp(-0.5*(lv+LOG2PI)) equals s/sqrt(pi), and
    exp(-0.5*w*(z-m)^2) = (sqrt(pi)/2)*DerivErf(s*z - s*m), so
      A_acc[i,d] = sum_j s * DerivErf(s*z - s*m) = 2*A[i,d].
    One ACT instruction per j-column batch (Derivative_Erf), one fused
    scalar_tensor_tensor accumulate per column on DVE/Pool.
  * Partition layout for the hot loop: p = (e,d), e = j-half, d = latent dim;
    free axis = all 256 local i. 1024 packed columns.
  * Final: out = (BETA-1)*(T_sum/B + K0) + KL_sum,
    K0 = -32*LOG2PI + 64*ln2  (host side, exact).
"""

import math
import sys

import numpy as np

if "/opt/trn_rl_repo" not in sys.path:
    sys.path.insert(0, "/opt/trn_rl_repo")

import concourse.bacc as bacc
import concourse.tile as tile
from concourse import mybir
from concourse.bass_utils import run_bass_kernel_spmd
from concourse.masks import make_identity

B, D, M = 2048, 64, 8
BL = B // M          # 256 local rows
NJT = B // 128       # 16 natural j-tiles
NCOL = B // 2        # 1024 packed columns (e-packing: j-halves on partitions)
KB = 8               # j-columns per DerivErf batch
F32 = mybir.dt.float32
BF16 = mybir.dt.bfloat16
LOG_2PI = math.log(2.0 * math.pi)
BETA = 6.0
K0 = -32.0 * LOG_2PI + 64.0 * math.log(2.0)

A = mybir.AluOpType
AF = mybir.ActivationFunctionType
AX = mybir.AxisListType


def _body(tc):
    nc = tc.nc
    kl_ext = nc.dram_tensor("kl", [BL, D], F32, kind="ExternalInput").ap()
    zm_ext = nc.dram_tensor("z_mean", [B, D], F32, kind="ExternalInput").ap()
    zlv_ext = nc.dram_tensor("z_logvar", [B, D], F32, kind="ExternalInput").ap()
    zs_ext = nc.dram_tensor("z_sampled", [BL, D], F32, kind="ExternalInput").ap()
    out_ext = nc.dram_tensor("out", [1, 2], F32, kind="ExternalOutput").ap()

    with (
        tc.tile_pool(name="cst", bufs=1) as cst,
        tc.tile_pool(name="mats", bufs=1) as mats,
        tc.tile_pool(name="ld", bufs=4) as ld,
        tc.tile_pool(name="yb", bufs=3) as yb,
        tc.tile_pool(name="db", bufs=3) as db,
    ):
        ident = cst.tile([128, 128], F32, tag="ident")
        make_identity(nc, ident)
        ones = cst.tile([128, 1], F32, tag="ones")
        nc.vector.memset(ones, 1.0)
        neghalf = cst.tile([128, 128], F32, tag="neghalf")
        nc.gpsimd.memset(neghalf, -0.5)

        # ---- load + transpose z_mean, z_logvar -> M_T/LV_T [64, 2048] ----
        m_t = mats.tile([64, B], F32, tag="m_t")
        lv_t = mats.tile([64, B], F32, tag="lv_t")
        z_t = mats.tile([64, BL], F32, tag="z_t")
        with tc.tile_pool(name="pst", bufs=4, space="PSUM") as pst:
            for t in range(NJT):
                nat = ld.tile([128, D], F32, tag="nat")
                nc.sync.dma_start(out=nat, in_=zm_ext[t * 128:(t + 1) * 128, :])
                ps = pst.tile([64, 128], F32, tag="tp")
                nc.tensor.transpose(ps, nat, ident)
                nc.vector.tensor_copy(out=m_t[0:64, t * 128:(t + 1) * 128], in_=ps)
            for t in range(NJT):
                nat = ld.tile([128, D], F32, tag="nat")
                nc.sync.dma_start(out=nat, in_=zlv_ext[t * 128:(t + 1) * 128, :])
                ps = pst.tile([64, 128], F32, tag="tp")
                nc.tensor.transpose(ps, nat, ident)
                nc.vector.tensor_copy(out=lv_t[0:64, t * 128:(t + 1) * 128], in_=ps)
            for t in range(2):
                nat = ld.tile([128, D], F32, tag="nat")
                nc.sync.dma_start(out=nat, in_=zs_ext[t * 128:(t + 1) * 128, :])
                ps = pst.tile([64, 128], F32, tag="tp")
                nc.tensor.transpose(ps, nat, ident)
                nc.vector.tensor_copy(out=z_t[0:64, t * 128:(t + 1) * 128], in_=ps)

        # ---- kl partial sum ----
        ks2 = mats.tile([128, 2], F32, tag="ks2")
        for t in range(2):
            klt = ld.tile([128, D], F32, tag="klt", bufs=2)
            nc.sync.dma_start(out=klt, in_=kl_ext[t * 128:(t + 1) * 128, :])
            nc.vector.tensor_reduce(out=ks2[:, t:t + 1], in_=klt, axis=AX.X, op=A.add)
        kss = mats.tile([128, 1], F32, tag="kss")
        nc.vector.tensor_reduce(out=kss, in_=ks2, axis=AX.X, op=A.add)

        # ---- prep params (T-layout, [64, 2048]) ----
        s_t = mats.tile([64, B], F32, tag="s_t")
        #  s = exp(-lv/2)/sqrt(2) = sqrt(w/2)
        bias_l2 = cst.tile([128, 1], F32, tag="bias_l2")
        nc.gpsimd.memset(bias_l2, -0.5 * math.log(2.0))
        nc.scalar.activation(out=s_t[0:64, :], in_=lv_t[0:64, :], func=AF.Exp,
                             bias=bias_l2[0:64, :], scale=-0.5)
        w_t = mats.tile([64, B], F32, tag="w_t")
        nc.vector.scalar_tensor_tensor(out=w_t[0:64, :], in0=s_t[0:64, :],
                                       scalar=2.0, in1=s_t[0:64, :],
                                       op0=A.mult, op1=A.mult)
        wm_t = mats.tile([64, B], F32, tag="wm_t")
        nc.vector.tensor_mul(out=wm_t[0:64, :], in0=w_t[0:64, :],
                             in1=m_t[0:64, :])
        t3 = mats.tile([64, B], F32, tag="t3")
        nc.gpsimd.tensor_mul(out=t3[0:64, :], in0=wm_t[0:64, :], in1=m_t[0:64, :])
        nc.gpsimd.tensor_add(out=t3[0:64, :], in0=t3[0:64, :], in1=lv_t[0:64, :])

        z2n_t = mats.tile([64, BL], F32, tag="z2n_t")
        nc.scalar.activation(out=z2n_t[0:64, :], in_=z_t[0:64, :], func=AF.Square,
                             bias=0.0, scale=1.0)
        nc.vector.tensor_scalar(out=z2n_t[0:64, :], in0=z2n_t[0:64, :],
                                scalar1=-0.5, scalar2=None, op0=A.mult)

        # ---- replicated bf16 tiles for the hot loop (partition = (h,d)) ----
        m_rep = mats.tile([128, B], BF16, tag="m_rep")
        nc.vector.tensor_copy(out=m_rep[0:64, :], in_=m_t[0:64, :])
        nc.sync.dma_start(out=m_rep[64:128, :], in_=m_rep[0:64, :])
        s_rep = mats.tile([128, B], BF16, tag="s_rep")
        nc.vector.tensor_copy(out=s_rep[0:64, :], in_=s_t[0:64, :])
        nc.sync.dma_start(out=s_rep[64:128, :], in_=s_rep[0:64, :])
        # z columns: partition p=(h,d) holds z[i = g + 128h, d] at column g
        zpk = mats.tile([128, 128], F32, tag="zpk")
        nc.sync.dma_start(out=zpk[0:64, :], in_=z_t[0:64, 0:128])
        nc.sync.dma_start(out=zpk[64:128, :], in_=z_t[0:64, 128:256])
        nzpk = mats.tile([128, 128], F32, tag="nzpk")
        nc.vector.tensor_scalar(out=nzpk, in0=zpk, scalar1=-1.0, scalar2=None,
                                op0=A.mult)

        # A[p=(h,d), g] = sum_j s * DerivErf(s*(z-m)) per (i=g+128h, d)
        a_mat = mats.tile([128, 128], F32, tag="a_mat")

        # ---- HOT LOOP: one group per z-column (i), j = full 2048 free ----
        # u = m - z_g  (sign-free under DerivErf), y = u*s, D = DerivErf(y),
        # A[:, g] = sum_j s*D  (affine_mul_reduce on DVE).
        NG = 128
        with (
            tc.tile_pool(name="ut", bufs=3) as up,
            tc.tile_pool(name="yt", bufs=3) as yp,
            tc.tile_pool(name="dt", bufs=3) as dp,
            tc.tile_pool(name="et", bufs=2) as ep,
        ):
            for g in range(NG):
                u_t = up.tile([128, B], BF16, tag="u")
                if g % 2 == 0:
                    nc.vector.tensor_scalar(out=u_t, in0=m_rep,
                                            scalar1=zpk[:, g:g + 1],
                                            scalar2=None, op0=A.subtract)
                else:
                    nc.scalar.activation(out=u_t, in_=m_rep, func=AF.Identity,
                                         bias=nzpk[:, g:g + 1], scale=1.0)
                y_t = yp.tile([128, B], BF16, tag="y")
                yeng = nc.vector if (g % 6 == 5) else nc.gpsimd
                yeng.tensor_tensor(out=y_t, in0=u_t, in1=s_rep, op=A.mult)
                d_t = dp.tile([128, B], BF16, tag="d")
                nc.scalar.activation(out=d_t, in_=y_t, func=AF.Derivative_Erf,
                                     bias=0.0, scale=1.0)
                e_t = ep.tile([128, B], BF16, tag="e")
                nc.vector.affine_mul_reduce(out=e_t,
                                            accum_out=a_mat[:, g:g + 1],
                                            in0=d_t, in1=s_rep,
                                            scale=1.0, bias=0.0)

        # ---- A epilogue: log then partition-reduce over d (per h-half) ----
        ln_a = mats.tile([128, 128], F32, tag="ln_a")
        nc.scalar.activation(out=ln_a, in_=a_mat, func=AF.Ln,
                             bias=0.0, scale=1.0)

        # ---- S' matmuls + logsumexp epilogue ----
        contrib = []
        with (
            tc.tile_pool(name="psp", bufs=1, space="PSUM") as psp,
            tc.tile_pool(name="psm", bufs=2, space="PSUM") as psm,
            tc.tile_pool(name="scr", bufs=2) as scr,
        ):
            for it in range(2):
                isl = slice(it * 128, (it + 1) * 128)
                sps = []
                for jb in range(4):
                    jsl = slice(jb * 512, (jb + 1) * 512)
                    sp = psp.tile([128, 512], F32, tag=f"sp{jb}")
                    nc.tensor.matmul(sp, lhsT=z2n_t[0:64, isl], rhs=w_t[0:64, jsl],
                                     start=True, stop=False)
                    nc.tensor.matmul(sp, lhsT=z_t[0:64, isl], rhs=wm_t[0:64, jsl],
                                     start=False, stop=False)
                    nc.tensor.matmul(sp, lhsT=neghalf[0:64, :], rhs=t3[0:64, jsl],
                                     start=False, stop=True)
                    sps.append(sp)
                mx4 = mats.tile([128, 4], F32, tag="mx4", bufs=2)
                for jb in range(4):
                    nc.vector.tensor_reduce(out=mx4[:, jb:jb + 1], in_=sps[jb],
                                            axis=AX.X, op=A.max)
                nmx = mats.tile([128, 1], F32, tag="nmx", bufs=2)
                nc.vector.tensor_reduce(out=nmx, in_=mx4, axis=AX.X, op=A.max,
                                        negate=True)
                es4 = mats.tile([128, 4], F32, tag="es4", bufs=2)
                for jb in range(4):
                    sc = scr.tile([128, 512], F32, tag="sc")
                    nc.scalar.activation(out=sc, in_=sps[jb], func=AF.Exp,
                                         bias=nmx, scale=1.0,
                                         accum_out=es4[:, jb:jb + 1])
                esum = mats.tile([128, 1], F32, tag="esum", bufs=2)
                nc.vector.tensor_reduce(out=esum, in_=es4, axis=AX.X, op=A.add)
                lqz = mats.tile([128, 1], F32, tag="lqz", bufs=2)
                nc.scalar.activation(out=lqz, in_=esum, func=AF.Ln,
                                     bias=0.0, scale=1.0)
                # lqz - P  (P via ones-matmul over d), both [128,1]
                # i-tile 0 <-> h=0 lives on partitions 0:64, i-tile 1 on 64:128
                psl = slice(it * 64, (it + 1) * 64)
                pps = psm.tile([128, 1], F32, tag="pp")
                nc.tensor.matmul(pps, lhsT=ln_a[psl, :], rhs=ones[psl, :],
                                 start=True, stop=True)
                ctr = mats.tile([128, 1], F32, tag="ctr", bufs=2)
                # ctr = (lqz + (-1)*mx4_max...) careful: lqz currently ln(esum);
                # full log_qz = lqz + mx ; contrib = lqz + mx - P
                mx = mats.tile([128, 1], F32, tag="mx", bufs=2)
                nc.vector.tensor_scalar(out=mx, in0=nmx, scalar1=-1.0,
                                        scalar2=None, op0=A.mult)
                nc.vector.tensor_add(out=lqz, in0=lqz, in1=mx)
                nc.vector.tensor_sub(out=ctr, in0=lqz, in1=pps)
                contrib.append(ctr)

            # ---- final scalars ----
            fps = psm.tile([1, 2], F32, tag="fps")
            nc.tensor.matmul(fps[0:1, 0:1], lhsT=contrib[0], rhs=ones,
                             start=True, stop=False)
            nc.tensor.matmul(fps[0:1, 0:1], lhsT=contrib[1], rhs=ones,
                             start=False, stop=True)
            nc.tensor.matmul(fps[0:1, 1:2], lhsT=kss, rhs=ones,
                             start=True, stop=True)
            out_sb = mats.tile([1, 2], F32, tag="out_sb")
            nc.vector.tensor_copy(out=out_sb[0:1, :], in_=fps[0:1, :])
            nc.sync.dma_start(out=out_ext, in_=out_sb[0:1, :])


_NC_CACHE = {}


def _get_nc():
    if "nc" not in _NC_CACHE:
        nc = bacc.Bacc("TRN2", target_bir_lowering=False, debug=False,
                       num_devices=M)
        with tile.TileContext(nc) as tc:
            _body(tc)
        nc.compile()
        _NC_CACHE["nc"] = nc
    return _NC_CACHE["nc"]


def kernel(kl, z_mean, z_logvar, z_sampled, _trace=False, _tmpdir=None):
    kl = np.ascontiguousarray(kl, dtype=np.float32)
    z_mean = np.ascontiguousarray(z_mean, dtype=np.float32)
    z_logvar = np.ascontiguousarray(z_logvar, dtype=np.float32)
    z_sampled = np.ascontiguousarray(z_sampled, dtype=np.float32)
    nc = _get_nc()
    in_maps = []
    for c in range(M):
        sl = slice(c * BL, (c + 1) * BL)
        in_maps.append({
            "kl": np.ascontiguousarray(kl[sl]),
            "z_mean": z_mean,
            "z_logvar": z_logvar,
            "z_sampled": np.ascontiguousarray(z_sampled[sl]),
        })
    res = run_bass_kernel_spmd(nc, in_maps, list(range(M)), trace=_trace,
                               tmpdir=_tmpdir)
    t_sum = 0.0
    kl_sum = 0.0
    for c in range(M):
        o = res.results[c]["out"]
        t_sum += float(o[0, 0])
        kl_sum += float(o[0, 1])
    val = (BETA - 1.0) * (t_sum / B + K0) + kl_sum
    out = np.float32(val)
    if _trace:
        return out, res
    return out



# revision 9
# speedup vs baseline: 2.6906x; 2.6906x over previous
"""BetaTCVAE loss kernel for 8 TRN2 NeuronCores (Bass/Tile). v2

Math
----
reference:  out = (BETA-1)*tc + sum(kl)
  lp[i,j,d] = -0.5*((z_i - m_j)^2 * w_jd + lv_jd + LOG2PI),  w = exp(-lv)
  log_qz_product[i] = sum_d logsumexp_j lp[i,j,d]
  log_qz[i]         = logsumexp_j sum_d lp[i,j,d]
  tc = mean_i(log_qz - log_qz_product)

Decomposition (per core c):
  lp[i,j,d] = f0(i,d)*g0(j,d) + f1(i,d)*g1(j,d) + 1*g2(j,d)
    f0 = -z^2/2, f1 = z;  g0 = w, g1 = w*m, g2 = -(w*m^2 + lv + LOG2PI)/2
  * A-part (d-sharded: 8 dims/core, all 2048 i): per (d, i-tile of 128):
    PSUM[128,2048] <- rank-3 f32r matmuls (TensorE);
    ACT Exp + accum_out (free-axis j-sum) -> A[i,(d,it)]; Ln; grand sum = Q_c.
  * S-part (i-sharded: 256 rows/core): S[i,j] = sum_d lp via 3 matmuls
    (contraction 64, f32r) per [128,512] PSUM tile; logsumexp_j;
    L_c = sum over local i of log_qz[i].
  * host: out = (BETA-1)*(sum_c L_c - sum_c Q_c)/B + sum(kl)
"""

import math
import sys

import numpy as np

if "/opt/trn_rl_repo" not in sys.path:
    sys.path.insert(0, "/opt/trn_rl_repo")

import concourse.bacc as bacc
import concourse.tile as tile
from concourse import mybir
from concourse.bass_utils import run_bass_kernel_spmd

B, D, M = 2048, 64, 8
DL = D // M          # 8 local dims (A-part shard)
BL = B // M          # 256 local rows (S-part shard)
NG = DL * (B // 128)  # 128 A-part groups
F32 = mybir.dt.float32
F32R = mybir.dt.float32r
BF16 = mybir.dt.bfloat16
LOG_2PI = math.log(2.0 * math.pi)
BETA = 6.0

A = mybir.AluOpType
AF = mybir.ActivationFunctionType
AX = mybir.AxisListType


def _body(tc):
    nc = tc.nc
    m_ext = nc.dram_tensor("m_t", [D, B], F32, kind="ExternalInput").ap()
    lv_ext = nc.dram_tensor("lv_t", [D, B], F32, kind="ExternalInput").ap()
    md_ext = nc.dram_tensor("md_t", [DL, B], F32, kind="ExternalInput").ap()
    lvd_ext = nc.dram_tensor("lvd_t", [DL, B], F32, kind="ExternalInput").ap()
    zd_ext = nc.dram_tensor("zd_t", [DL, B], F32, kind="ExternalInput").ap()
    zi_ext = nc.dram_tensor("zi_t", [D, BL], F32, kind="ExternalInput").ap()
    kl_ext = nc.dram_tensor("kl", [BL, D], F32, kind="ExternalInput").ap()
    out_ext = nc.dram_tensor("out", [1, 4], F32, kind="ExternalOutput").ap()

    with (
        tc.tile_pool(name="mats", bufs=1) as mats,
        tc.tile_pool(name="ld", bufs=2) as ld,
    ):
        ones = mats.tile([128, 1], F32, tag="ones")
        nc.vector.memset(ones, 1.0)
        ones_sf = mats.tile([D, 128], F32, tag="ones_sf")
        nc.gpsimd.memset(ones_sf, 1.0)
        ones_s = mats.tile([D, 128], F32R, tag="ones_s")
        nc.vector.tensor_copy(out=ones_s, in_=ones_sf)

        # ---------------- prep (scoped; freed before hot loop) ----------------
        zf = [mats.tile([67, B], F32R, tag=f"zf{t}", name=f"zf{t}")
              for t in range(3)]
        gf = [mats.tile([67, B], F32R, tag=f"gf{t}", name=f"gf{t}")
              for t in range(3)]
        w_t = mats.tile([D, B], F32R, tag="w_t")
        wm_t = mats.tile([D, B], F32R, tag="wm_t")
        c_t = mats.tile([D, B], F32R, tag="c_t")
        zi_r = mats.tile([D, BL], F32R, tag="zi_r")
        z2ni = mats.tile([D, BL], F32R, tag="z2ni")

        with tc.tile_pool(name="prep", bufs=1) as prep:
            m_t = prep.tile([D, B], F32, tag="m_t")
            nc.sync.dma_start(out=m_t, in_=m_ext)
            lv_t = prep.tile([D, B], F32, tag="lv_t")
            nc.sync.dma_start(out=lv_t, in_=lv_ext)
            zi_t = prep.tile([D, BL], F32, tag="zi_t")
            nc.sync.dma_start(out=zi_t, in_=zi_ext)
            md_t = prep.tile([DL, B], F32, tag="md_t")
            nc.sync.dma_start(out=md_t, in_=md_ext)
            lvd_t = prep.tile([DL, B], F32, tag="lvd_t")
            nc.sync.dma_start(out=lvd_t, in_=lvd_ext)

            # A-part feature tiles. Matmul operands must sit at base
            # partition 0/32/64, so the 8 per-d rank-3 feature groups are
            # scattered over 3 tiles x 3 bases: d -> (tile d//3, base
            # 32*(d%3)), rows base+{0,1,2}.
            # zf rows: {-z^2/2, z, 1};  gf rows: {w, w*m, c}.
            zd_t = prep.tile([DL, B], F32, tag="zd_t")
            nc.sync.dma_start(out=zd_t, in_=zd_ext)
            zd_r = prep.tile([DL, B], F32R, tag="zd_r")
            nc.vector.tensor_copy(out=zd_r, in_=zd_t)
            z2nd = prep.tile([DL, B], F32R, tag="z2nd")
            nc.vector.scalar_tensor_tensor(out=z2nd, in0=zd_t, scalar=-0.5,
                                           in1=zd_t, op0=A.mult, op1=A.mult)
            ones8 = prep.tile([DL, B], F32, tag="ones8")
            nc.gpsimd.memset(ones8, 1.0)
            ones8_r = prep.tile([DL, B], F32R, tag="ones8_r")
            nc.vector.tensor_copy(out=ones8_r, in_=ones8)
            wd_r = prep.tile([DL, B], F32R, tag="wd_r")
            nc.scalar.activation(out=wd_r, in_=lvd_t, func=AF.Exp,
                                 bias=0.0, scale=-1.0)
            wmd_r = prep.tile([DL, B], F32R, tag="wmd_r")
            nc.vector.tensor_tensor(out=wmd_r, in0=wd_r.bitcast(F32),
                                    in1=md_t, op=A.mult)
            qd = prep.tile([DL, B], F32, tag="qd")
            nc.gpsimd.tensor_tensor(out=qd, in0=wmd_r.bitcast(F32),
                                    in1=md_t, op=A.mult)
            nc.gpsimd.tensor_tensor(out=qd, in0=qd, in1=lvd_t, op=A.add)
            cd = prep.tile([DL, B], F32R, tag="cd")
            nc.vector.tensor_scalar(out=cd, in0=qd, scalar1=LOG_2PI,
                                    scalar2=-0.5, op0=A.add, op1=A.mult)

            for d in range(DL):
                t, base = d // 3, 32 * (d % 3)
                nc.sync.dma_start(out=zf[t][base:base + 1, :],
                                  in_=z2nd[d:d + 1, :])
                nc.sync.dma_start(out=zf[t][base + 1:base + 2, :],
                                  in_=zd_r[d:d + 1, :])
                nc.sync.dma_start(out=zf[t][base + 2:base + 3, :],
                                  in_=ones8_r[d:d + 1, :])
                nc.sync.dma_start(out=gf[t][base:base + 1, :],
                                  in_=wd_r[d:d + 1, :])
                nc.sync.dma_start(out=gf[t][base + 1:base + 2, :],
                                  in_=wmd_r[d:d + 1, :])
                nc.sync.dma_start(out=gf[t][base + 2:base + 3, :],
                                  in_=cd[d:d + 1, :])

            # full params for the S-part
            nc.scalar.activation(out=w_t, in_=lv_t, func=AF.Exp, bias=0.0,
                                 scale=-1.0)
            nc.vector.tensor_tensor(out=wm_t, in0=w_t.bitcast(F32),
                                    in1=m_t, op=A.mult)
            qf = prep.tile([D, B], F32, tag="qf")
            nc.gpsimd.tensor_tensor(out=qf, in0=wm_t.bitcast(F32),
                                    in1=m_t, op=A.mult)
            nc.gpsimd.tensor_tensor(out=qf, in0=qf, in1=lv_t, op=A.add)
            nc.vector.tensor_scalar(out=c_t, in0=qf, scalar1=LOG_2PI,
                                    scalar2=-0.5, op0=A.add, op1=A.mult)
            # local-i z features [64, 256]
            nc.vector.tensor_copy(out=zi_r, in_=zi_t)
            nc.vector.scalar_tensor_tensor(out=z2ni, in0=zi_t, scalar=-0.5,
                                           in1=zi_t, op0=A.mult, op1=A.mult)

        # ---------------- kl partial sum ----------------
        ks2 = mats.tile([128, 2], F32, tag="ks2")
        for t in range(2):
            klt = ld.tile([128, D], F32, tag="klt")
            nc.sync.dma_start(out=klt, in_=kl_ext[t * 128:(t + 1) * 128, :])
            nc.vector.tensor_reduce(out=ks2[:, t:t + 1], in_=klt, axis=AX.X,
                                    op=A.add)
        kss = mats.tile([128, 1], F32, tag="kss")
        nc.vector.tensor_reduce(out=kss, in_=ks2, axis=AX.X, op=A.add)

        # ---------------- S-part: log_qz over local i ----------------
        contrib = []
        with (
            tc.tile_pool(name="psS", bufs=1, space="PSUM") as psS,
            tc.tile_pool(name="scr", bufs=2) as scr,
        ):
            for it in range(2):
                isl = slice(it * 128, (it + 1) * 128)
                sps = []
                for jb in range(4):
                    jsl = slice(jb * 512, (jb + 1) * 512)
                    sp = psS.tile([128, 512], F32, tag=f"sp{jb}")
                    nc.tensor.matmul(sp, lhsT=z2ni[:, isl], rhs=w_t[:, jsl],
                                     start=True, stop=False)
                    nc.tensor.matmul(sp, lhsT=zi_r[:, isl], rhs=wm_t[:, jsl],
                                     start=False, stop=False)
                    nc.tensor.matmul(sp, lhsT=ones_s, rhs=c_t[:, jsl],
                                     start=False, stop=True)
                    sps.append(sp)
                mx4 = mats.tile([128, 4], F32, tag="mx4", bufs=2)
                for jb in range(4):
                    nc.vector.tensor_reduce(out=mx4[:, jb:jb + 1], in_=sps[jb],
                                            axis=AX.X, op=A.max)
                nmx = mats.tile([128, 1], F32, tag="nmx", bufs=2)
                nc.vector.tensor_reduce(out=nmx, in_=mx4, axis=AX.X, op=A.max,
                                        negate=True)
                es4 = mats.tile([128, 4], F32, tag="es4", bufs=2)
                for jb in range(4):
                    sc = scr.tile([128, 512], BF16, tag="sc")
                    nc.scalar.activation(out=sc, in_=sps[jb], func=AF.Exp,
                                         bias=nmx, scale=1.0,
                                         accum_out=es4[:, jb:jb + 1])
                esum = mats.tile([128, 1], F32, tag="esum", bufs=2)
                nc.vector.tensor_reduce(out=esum, in_=es4, axis=AX.X, op=A.add)
                lqz = mats.tile([128, 1], F32, tag="lqz", bufs=2)
                nc.scalar.activation(out=lqz, in_=esum, func=AF.Ln,
                                     bias=0.0, scale=1.0)
                # log_qz = ln(esum) + mx = ln(esum) - nmx
                ctr = mats.tile([128, 1], F32, tag="ctr", bufs=2)
                nc.vector.tensor_tensor(out=ctr, in0=lqz, in1=nmx,
                                        op=A.subtract)
                contrib.append(ctr)

        # ---------------- A-part hot loop ----------------
        a_acc = mats.tile([128, NG], F32, tag="a_acc")
        with (
            tc.tile_pool(name="psA", bufs=2, space="PSUM") as psA,
            tc.tile_pool(name="eb", bufs=2) as eb,
        ):
            for d in range(DL):
                t, base = d // 3, 32 * (d % 3)
                zfd = zf[t][base:base + 3, :]
                gfd = gf[t][base:base + 3, :]
                for it in range(B // 128):
                    g = d * (B // 128) + it
                    ps = psA.tile([128, B], F32, tag="T")
                    for jq in range(4):
                        jsl = slice(jq * 512, (jq + 1) * 512)
                        nc.tensor.matmul(
                            ps[:, jsl],
                            lhsT=zfd[:, it * 128:(it + 1) * 128],
                            rhs=gfd[:, jsl],
                            start=True, stop=True)
                    et = eb.tile([128, B], BF16, tag="e")
                    nc.scalar.activation(out=et, in_=ps, func=AF.Exp,
                                         bias=0.0, scale=1.0,
                                         accum_out=a_acc[:, g:g + 1])

        # ---------------- epilogue ----------------
        ln_a = mats.tile([128, NG], F32, tag="ln_a")
        nc.scalar.activation(out=ln_a, in_=a_acc, func=AF.Ln, bias=0.0,
                             scale=1.0)
        qrow = mats.tile([128, 1], F32, tag="qrow")
        nc.vector.tensor_reduce(out=qrow, in_=ln_a, axis=AX.X, op=A.add)

        with tc.tile_pool(name="psF", bufs=1, space="PSUM") as psF:
            fps = psF.tile([1, 4], F32, tag="fps")
            nc.tensor.matmul(fps[0:1, 0:1], lhsT=contrib[0], rhs=ones,
                             start=True, stop=False)
            nc.tensor.matmul(fps[0:1, 0:1], lhsT=contrib[1], rhs=ones,
                             start=False, stop=True)
            nc.tensor.matmul(fps[0:1, 1:2], lhsT=qrow, rhs=ones,
                             start=True, stop=True)
            nc.tensor.matmul(fps[0:1, 2:3], lhsT=kss, rhs=ones,
                             start=True, stop=True)
            out_sb = mats.tile([1, 4], F32, tag="out_sb")
            nc.vector.tensor_copy(out=out_sb[0:1, :], in_=fps[0:1, :])
            nc.sync.dma_start(out=out_ext, in_=out_sb[0:1, :])


_NC_CACHE = {}


def _get_nc():
    if "nc" not in _NC_CACHE:
        nc = bacc.Bacc("TRN2", target_bir_lowering=False, debug=False,
                       num_devices=M)
        with tile.TileContext(nc) as tc:
            _body(tc)
        nc.compile()
        _NC_CACHE["nc"] = nc
    return _NC_CACHE["nc"]


def kernel(kl, z_mean, z_logvar, z_sampled, _trace=False, _tmpdir=None):
    kl = np.ascontiguousarray(kl, dtype=np.float32)
    mT = np.ascontiguousarray(np.asarray(z_mean, dtype=np.float32).T)
    lvT = np.ascontiguousarray(np.asarray(z_logvar, dtype=np.float32).T)
    zT = np.ascontiguousarray(np.asarray(z_sampled, dtype=np.float32).T)
    nc = _get_nc()
    in_maps = []
    for c in range(M):
        dsl = slice(c * DL, (c + 1) * DL)
        isl = slice(c * BL, (c + 1) * BL)
        in_maps.append({
            "m_t": mT,
            "lv_t": lvT,
            "md_t": np.ascontiguousarray(mT[dsl]),
            "lvd_t": np.ascontiguousarray(lvT[dsl]),
            "zd_t": np.ascontiguousarray(zT[dsl]),
            "zi_t": np.ascontiguousarray(zT[:, isl]),
            "kl": np.ascontiguousarray(kl[isl]),
        })
    res = run_bass_kernel_spmd(nc, in_maps, list(range(M)), trace=_trace,
                               tmpdir=_tmpdir)
    l_sum = 0.0
    q_sum = 0.0
    kl_sum = 0.0
    for c in range(M):
        o = res.results[c]["out"]
        l_sum += float(o[0, 0])
        q_sum += float(o[0, 1])
        kl_sum += float(o[0, 2])
    val = (BETA - 1.0) * ((l_sum - q_sum) / B) + kl_sum
    out = np.float32(val)
    if _trace:
        return out, res
    return out


# revision 10
# speedup vs baseline: 2.8051x; 1.0425x over previous
"""BetaTCVAE loss kernel for 8 TRN2 NeuronCores (Bass/Tile). v2

Math
----
reference:  out = (BETA-1)*tc + sum(kl)
  lp[i,j,d] = -0.5*((z_i - m_j)^2 * w_jd + lv_jd + LOG2PI),  w = exp(-lv)
  log_qz_product[i] = sum_d logsumexp_j lp[i,j,d]
  log_qz[i]         = logsumexp_j sum_d lp[i,j,d]
  tc = mean_i(log_qz - log_qz_product)

Decomposition (per core c):
  lp[i,j,d] = f0(i,d)*g0(j,d) + f1(i,d)*g1(j,d) + 1*g2(j,d)
    f0 = -z^2/2, f1 = z;  g0 = w, g1 = w*m, g2 = -(w*m^2 + lv + LOG2PI)/2
  * A-part (d-sharded: 8 dims/core, all 2048 i): per (d, i-tile of 128):
    PSUM[128,2048] <- rank-3 f32r matmuls (TensorE);
    ACT Exp + accum_out (free-axis j-sum) -> A[i,(d,it)]; Ln; grand sum = Q_c.
  * S-part (i-sharded: 256 rows/core): S[i,j] = sum_d lp via 3 matmuls
    (contraction 64, f32r) per [128,512] PSUM tile; logsumexp_j;
    L_c = sum over local i of log_qz[i].
  * host: out = (BETA-1)*(sum_c L_c - sum_c Q_c)/B + sum(kl)
"""

import math
import sys

import numpy as np

if "/opt/trn_rl_repo" not in sys.path:
    sys.path.insert(0, "/opt/trn_rl_repo")

import concourse.bacc as bacc
import concourse.tile as tile
from concourse import mybir
from concourse.bass_utils import run_bass_kernel_spmd

B, D, M = 2048, 64, 8
DL = D // M          # 8 local dims (A-part shard)
BL = B // M          # 256 local rows (S-part shard)
NG = DL * (B // 128)  # 128 A-part groups
F32 = mybir.dt.float32
F32R = mybir.dt.float32r
BF16 = mybir.dt.bfloat16
LOG_2PI = math.log(2.0 * math.pi)
BETA = 6.0

A = mybir.AluOpType
AF = mybir.ActivationFunctionType
AX = mybir.AxisListType


def _body(tc):
    nc = tc.nc
    m_ext = nc.dram_tensor("m_t", [D, B], F32, kind="ExternalInput").ap()
    lv_ext = nc.dram_tensor("lv_t", [D, B], F32, kind="ExternalInput").ap()
    md_ext = nc.dram_tensor("md_t", [DL, B], F32, kind="ExternalInput").ap()
    lvd_ext = nc.dram_tensor("lvd_t", [DL, B], F32, kind="ExternalInput").ap()
    zd_ext = nc.dram_tensor("zd_t", [DL, B], F32, kind="ExternalInput").ap()
    zi_ext = nc.dram_tensor("zi_t", [D, BL], F32, kind="ExternalInput").ap()
    kl_ext = nc.dram_tensor("kl", [BL, D], F32, kind="ExternalInput").ap()
    out_ext = nc.dram_tensor("out", [1, 4], F32, kind="ExternalOutput").ap()

    with (
        tc.tile_pool(name="mats", bufs=1) as mats,
        tc.tile_pool(name="ld", bufs=2) as ld,
    ):
        ones = mats.tile([128, 1], F32, tag="ones")
        nc.vector.memset(ones, 1.0)
        ones_sf = mats.tile([D, 128], F32, tag="ones_sf")
        nc.gpsimd.memset(ones_sf, 1.0)
        ones_s = mats.tile([D, 128], F32R, tag="ones_s")
        nc.vector.tensor_copy(out=ones_s, in_=ones_sf)

        # ---------------- prep (scoped; freed before hot loop) ----------------
        zf = [mats.tile([67, B], BF16, tag=f"zf{t}", name=f"zf{t}")
              for t in range(3)]
        gf = [mats.tile([67, B], BF16, tag=f"gf{t}", name=f"gf{t}")
              for t in range(3)]
        w_t = mats.tile([D, B], F32R, tag="w_t")
        wm_t = mats.tile([D, B], F32R, tag="wm_t")
        c_t = mats.tile([D, B], F32R, tag="c_t")
        zi_r = mats.tile([D, BL], F32R, tag="zi_r")
        z2ni = mats.tile([D, BL], F32R, tag="z2ni")

        with tc.tile_pool(name="prep", bufs=1) as prep:
            m_t = prep.tile([D, B], F32, tag="m_t")
            nc.sync.dma_start(out=m_t, in_=m_ext)
            lv_t = prep.tile([D, B], F32, tag="lv_t")
            nc.sync.dma_start(out=lv_t, in_=lv_ext)
            zi_t = prep.tile([D, BL], F32, tag="zi_t")
            nc.sync.dma_start(out=zi_t, in_=zi_ext)
            md_t = prep.tile([DL, B], F32, tag="md_t")
            nc.sync.dma_start(out=md_t, in_=md_ext)
            lvd_t = prep.tile([DL, B], F32, tag="lvd_t")
            nc.sync.dma_start(out=lvd_t, in_=lvd_ext)

            # A-part feature tiles. Matmul operands must sit at base
            # partition 0/32/64, so the 8 per-d rank-3 feature groups are
            # scattered over 3 tiles x 3 bases: d -> (tile d//3, base
            # 32*(d%3)), rows base+{0,1,2}.
            # zf rows: {-z^2/2, z, 1};  gf rows: {w, w*m, c}.
            zd_t = prep.tile([DL, B], F32, tag="zd_t")
            nc.sync.dma_start(out=zd_t, in_=zd_ext)
            zd_r = prep.tile([DL, B], BF16, tag="zd_r")
            nc.vector.tensor_copy(out=zd_r, in_=zd_t)
            z2nd = prep.tile([DL, B], BF16, tag="z2nd")
            nc.vector.scalar_tensor_tensor(out=z2nd, in0=zd_t, scalar=-0.5,
                                           in1=zd_t, op0=A.mult, op1=A.mult)
            ones8_r = prep.tile([DL, B], BF16, tag="ones8_r")
            nc.gpsimd.memset(ones8_r, 1.0)
            wd = prep.tile([DL, B], F32, tag="wd")
            nc.scalar.activation(out=wd, in_=lvd_t, func=AF.Exp,
                                 bias=0.0, scale=-1.0)
            wd_r = prep.tile([DL, B], BF16, tag="wd_r")
            nc.vector.tensor_copy(out=wd_r, in_=wd)
            wmd = prep.tile([DL, B], F32, tag="wmd")
            nc.vector.tensor_tensor(out=wmd, in0=wd, in1=md_t, op=A.mult)
            wmd_r = prep.tile([DL, B], BF16, tag="wmd_r")
            nc.vector.tensor_copy(out=wmd_r, in_=wmd)
            qd = prep.tile([DL, B], F32, tag="qd")
            nc.gpsimd.tensor_tensor(out=qd, in0=wmd, in1=md_t, op=A.mult)
            nc.gpsimd.tensor_tensor(out=qd, in0=qd, in1=lvd_t, op=A.add)
            cd = prep.tile([DL, B], BF16, tag="cd")
            nc.vector.tensor_scalar(out=cd, in0=qd, scalar1=LOG_2PI,
                                    scalar2=-0.5, op0=A.add, op1=A.mult)

            for d in range(DL):
                t, base = d // 3, 32 * (d % 3)
                nc.sync.dma_start(out=zf[t][base:base + 1, :],
                                  in_=z2nd[d:d + 1, :])
                nc.sync.dma_start(out=zf[t][base + 1:base + 2, :],
                                  in_=zd_r[d:d + 1, :])
                nc.sync.dma_start(out=zf[t][base + 2:base + 3, :],
                                  in_=ones8_r[d:d + 1, :])
                nc.sync.dma_start(out=gf[t][base:base + 1, :],
                                  in_=wd_r[d:d + 1, :])
                nc.sync.dma_start(out=gf[t][base + 1:base + 2, :],
                                  in_=wmd_r[d:d + 1, :])
                nc.sync.dma_start(out=gf[t][base + 2:base + 3, :],
                                  in_=cd[d:d + 1, :])

            # full params for the S-part
            nc.scalar.activation(out=w_t, in_=lv_t, func=AF.Exp, bias=0.0,
                                 scale=-1.0)
            nc.vector.tensor_tensor(out=wm_t, in0=w_t.bitcast(F32),
                                    in1=m_t, op=A.mult)
            qf = prep.tile([D, B], F32, tag="qf")
            nc.gpsimd.tensor_tensor(out=qf, in0=wm_t.bitcast(F32),
                                    in1=m_t, op=A.mult)
            nc.gpsimd.tensor_tensor(out=qf, in0=qf, in1=lv_t, op=A.add)
            nc.vector.tensor_scalar(out=c_t, in0=qf, scalar1=LOG_2PI,
                                    scalar2=-0.5, op0=A.add, op1=A.mult)
            # local-i z features [64, 256]
            nc.vector.tensor_copy(out=zi_r, in_=zi_t)
            nc.vector.scalar_tensor_tensor(out=z2ni, in0=zi_t, scalar=-0.5,
                                           in1=zi_t, op0=A.mult, op1=A.mult)

        # ---------------- kl partial sum ----------------
        ks2 = mats.tile([128, 2], F32, tag="ks2")
        for t in range(2):
            klt = ld.tile([128, D], F32, tag="klt")
            nc.sync.dma_start(out=klt, in_=kl_ext[t * 128:(t + 1) * 128, :])
            nc.vector.tensor_reduce(out=ks2[:, t:t + 1], in_=klt, axis=AX.X,
                                    op=A.add)
        kss = mats.tile([128, 1], F32, tag="kss")
        nc.vector.tensor_reduce(out=kss, in_=ks2, axis=AX.X, op=A.add)

        # ---------------- S-part: log_qz over local i ----------------
        contrib = []
        with (
            tc.tile_pool(name="psS", bufs=1, space="PSUM") as psS,
            tc.tile_pool(name="scr", bufs=2) as scr,
        ):
            for it in range(2):
                isl = slice(it * 128, (it + 1) * 128)
                sps = []
                for jb in range(4):
                    jsl = slice(jb * 512, (jb + 1) * 512)
                    sp = psS.tile([128, 512], F32, tag=f"sp{jb}")
                    nc.tensor.matmul(sp, lhsT=z2ni[:, isl], rhs=w_t[:, jsl],
                                     start=True, stop=False)
                    nc.tensor.matmul(sp, lhsT=zi_r[:, isl], rhs=wm_t[:, jsl],
                                     start=False, stop=False)
                    nc.tensor.matmul(sp, lhsT=ones_s, rhs=c_t[:, jsl],
                                     start=False, stop=True)
                    sps.append(sp)
                mx4 = mats.tile([128, 4], F32, tag="mx4", bufs=2)
                for jb in range(4):
                    nc.vector.tensor_reduce(out=mx4[:, jb:jb + 1], in_=sps[jb],
                                            axis=AX.X, op=A.max)
                nmx = mats.tile([128, 1], F32, tag="nmx", bufs=2)
                nc.vector.tensor_reduce(out=nmx, in_=mx4, axis=AX.X, op=A.max,
                                        negate=True)
                es4 = mats.tile([128, 4], F32, tag="es4", bufs=2)
                for jb in range(4):
                    sc = scr.tile([128, 512], BF16, tag="sc")
                    nc.scalar.activation(out=sc, in_=sps[jb], func=AF.Exp,
                                         bias=nmx, scale=1.0,
                                         accum_out=es4[:, jb:jb + 1])
                esum = mats.tile([128, 1], F32, tag="esum", bufs=2)
                nc.vector.tensor_reduce(out=esum, in_=es4, axis=AX.X, op=A.add)
                lqz = mats.tile([128, 1], F32, tag="lqz", bufs=2)
                nc.scalar.activation(out=lqz, in_=esum, func=AF.Ln,
                                     bias=0.0, scale=1.0)
                # log_qz = ln(esum) + mx = ln(esum) - nmx
                ctr = mats.tile([128, 1], F32, tag="ctr", bufs=2)
                nc.vector.tensor_tensor(out=ctr, in0=lqz, in1=nmx,
                                        op=A.subtract)
                contrib.append(ctr)

        # ---------------- A-part hot loop ----------------
        a_acc = mats.tile([128, NG], F32, tag="a_acc")
        with (
            tc.tile_pool(name="psA", bufs=2, space="PSUM") as psA,
            tc.tile_pool(name="eb", bufs=2) as eb,
        ):
            for d in range(DL):
                t, base = d // 3, 32 * (d % 3)
                zfd = zf[t][base:base + 3, :]
                gfd = gf[t][base:base + 3, :]
                for it in range(B // 128):
                    g = d * (B // 128) + it
                    ps = psA.tile([128, B], F32, tag="T")
                    for jq in range(4):
                        jsl = slice(jq * 512, (jq + 1) * 512)
                        nc.tensor.matmul(
                            ps[:, jsl],
                            lhsT=zfd[:, it * 128:(it + 1) * 128],
                            rhs=gfd[:, jsl],
                            start=True, stop=True)
                    et = eb.tile([128, B], BF16, tag="e")
                    nc.scalar.activation(out=et, in_=ps, func=AF.Exp,
                                         bias=0.0, scale=1.0,
                                         accum_out=a_acc[:, g:g + 1])

        # ---------------- epilogue ----------------
        ln_a = mats.tile([128, NG], F32, tag="ln_a")
        nc.scalar.activation(out=ln_a, in_=a_acc, func=AF.Ln, bias=0.0,
                             scale=1.0)
        qrow = mats.tile([128, 1], F32, tag="qrow")
        nc.vector.tensor_reduce(out=qrow, in_=ln_a, axis=AX.X, op=A.add)

        with tc.tile_pool(name="psF", bufs=1, space="PSUM") as psF:
            fps = psF.tile([1, 4], F32, tag="fps")
            nc.tensor.matmul(fps[0:1, 0:1], lhsT=contrib[0], rhs=ones,
                             start=True, stop=False)
            nc.tensor.matmul(fps[0:1, 0:1], lhsT=contrib[1], rhs=ones,
                             start=False, stop=True)
            nc.tensor.matmul(fps[0:1, 1:2], lhsT=qrow, rhs=ones,
                             start=True, stop=True)
            nc.tensor.matmul(fps[0:1, 2:3], lhsT=kss, rhs=ones,
                             start=True, stop=True)
            out_sb = mats.tile([1, 4], F32, tag="out_sb")
            nc.vector.tensor_copy(out=out_sb[0:1, :], in_=fps[0:1, :])
            nc.sync.dma_start(out=out_ext, in_=out_sb[0:1, :])


_NC_CACHE = {}


def _get_nc():
    if "nc" not in _NC_CACHE:
        nc = bacc.Bacc("TRN2", target_bir_lowering=False, debug=False,
                       num_devices=M)
        with tile.TileContext(nc) as tc:
            _body(tc)
        nc.compile()
        _NC_CACHE["nc"] = nc
    return _NC_CACHE["nc"]


def kernel(kl, z_mean, z_logvar, z_sampled, _trace=False, _tmpdir=None):
    kl = np.ascontiguousarray(kl, dtype=np.float32)
    mT = np.ascontiguousarray(np.asarray(z_mean, dtype=np.float32).T)
    lvT = np.ascontiguousarray(np.asarray(z_logvar, dtype=np.float32).T)
    zT = np.ascontiguousarray(np.asarray(z_sampled, dtype=np.float32).T)
    nc = _get_nc()
    in_maps = []
    for c in range(M):
        dsl = slice(c * DL, (c + 1) * DL)
        isl = slice(c * BL, (c + 1) * BL)
        in_maps.append({
            "m_t": mT,
            "lv_t": lvT,
            "md_t": np.ascontiguousarray(mT[dsl]),
            "lvd_t": np.ascontiguousarray(lvT[dsl]),
            "zd_t": np.ascontiguousarray(zT[dsl]),
            "zi_t": np.ascontiguousarray(zT[:, isl]),
            "kl": np.ascontiguousarray(kl[isl]),
        })
    res = run_bass_kernel_spmd(nc, in_maps, list(range(M)), trace=_trace,
                               tmpdir=_tmpdir)
    l_sum = 0.0
    q_sum = 0.0
    kl_sum = 0.0
    for c in range(M):
        o = res.results[c]["out"]
        l_sum += float(o[0, 0])
        q_sum += float(o[0, 1])
        kl_sum += float(o[0, 2])
    val = (BETA - 1.0) * ((l_sum - q_sum) / B) + kl_sum
    out = np.float32(val)
    if _trace:
        return out, res
    return out
